# revision 11
# baseline (speedup 1.0000x reference)
"""LSTM decoder kernel for Trainium2 (8 NeuronCores, pure data parallel).

Problem: 25-step autoregressive LSTM decode, BATCH=262144, POSE=16, H=32.
  reference: per step  gates = x@W_ih.T + h@W_hh.T + b;  i,f,g,o = split(gates)
             c = sig(f)*c + sig(i)*tanh(g); h = sig(o)*tanh(c); x = h@W_out.T + b_out

Design (v3) — ACT (ScalarE) is the hard bottleneck: every gate element must
pass through one ACTIVATE from PSUM (a DVE-side sigmoid would first need a
PSUM->SBUF copy that costs more than the ACT itself).  Gate activations
therefore stay on ACT; everything else is trimmed:
  * Projection folded into the recurrence (W_eff = W_ih@W_out + W_hh); pose
    outputs recovered on the HOST from the streamed h-sequence (bf16).
  * Strip layout: state [128 = 4 strips x 32 feats, 8192 cols]; block-diag
    [128,128] stationary per gate type -> 4x N=512 matmuls per (type, block).
  * Per pair (2 blocks = 4096 cols): 8 gate ACTs FD=2048 (PSUM->SBUF bf16,
    per-partition bias), cell update as 3 TT + 1 TT at FD=4096 (halves DVE
    instruction overhead vs FD=2048).
  * tanh(c): 62.5% on DVE (pair0 fully + 1024 cols of pair1) via a clamped
    deg-7 odd poly using fused tensor_scalar (clamp = min+max in ONE op,
    c7*t+gamma in ONE op) and scalar_tensor_tensor Horner ((t+a)*t etc), 3
    stt + 1 TT + 2 TS per tile; the rest on ACT (Tanh, FD 1024/2048).
  * dtypes: matmuls/state bf16, PSUM f32; end-to-end rel l2 ~ 6e-3.
"""

import numpy as np
import ml_dtypes

bf16 = ml_dtypes.bfloat16

H = 32
PD = 16
SEQ = 25
BATCH = 262144
NCORES = 8

B_LOC = BATCH // NCORES          # 32768 rows per core
NB = 512                         # cols per group-chunk (one strip-group)
GROUPS = B_LOC // (4 * NB)       # 16 groups of 2048 rows
BLOCKS = 4                       # 4 blocks per step => FD=2048 each
FD = (GROUPS // BLOCKS) * NB     # 2048
FD2 = 2 * FD                     # 4096 (one pair)
CALL = GROUPS * NB               # 8192 state cols per core

USE_STT = False                  # stt measures 1x on HW — TS+TT chain wins

# tanh(c) ~ xc*(q + delta), q = (u + beta)*z, u = (t + alpha)*t,
# z = c7*t + gamma, t = xc^2, xc = clamp(c, +-XCLAMP).  deg-7 minimax-ish fit
# on [0, 2.5].
XCLAMP = 2.5
_C1, _C3, _C5, _C7 = 0.9700571610631279, -0.23796964748377925, \
    0.03875110981187037, -0.0024742524169626846
ALPHA = -8.0
GAMMA = _C5 - ALPHA * _C7
BETA = (_C3 - ALPHA * GAMMA) / _C7
DELTA = _C1 - BETA * GAMMA


def _f32(x):
    return np.ascontiguousarray(np.asarray(x, dtype=np.float32))


def _blkdiag(Wty, in_dim):
    """[32(out k), in_dim(m)] gate-type slice -> [128,128] block-diag lhsT.

    lhsT[32x+m, 32x+k] = Wty[k, m] for m < in_dim (zeros elsewhere).
    """
    out = np.zeros((128, 128), np.float32)
    blk = np.zeros((32, 32), np.float32)
    blk[:in_dim, :] = Wty.T[:in_dim, :]
    for x in range(4):
        out[32 * x : 32 * x + 32, 32 * x : 32 * x + 32] = blk
    return out


def prep_weights(W_ih, W_hh, b_ih, b_hh, W_out, b_out):
    W_ih, W_hh, b_ih, b_hh, W_out, b_out = map(
        _f32, (W_ih, W_hh, b_ih, b_hh, W_out, b_out)
    )
    b1 = b_ih + b_hh
    W_eff = W_ih @ W_out + W_hh            # [4H, H]
    b_eff = b1 + W_ih @ b_out

    def pack4(W, in_dim):
        # [128, 512]: cols 128*ty..+128 = block-diag lhsT for gate type ty
        return np.ascontiguousarray(np.concatenate(
            [_blkdiag(W[32 * ty : 32 * ty + 32, :], in_dim) for ty in range(4)],
            axis=1).astype(bf16))

    weff = pack4(W_eff, H)
    whh = pack4(W_hh, H)
    wih = pack4(W_ih, PD)

    bias = np.zeros((128, 8), np.float32)
    for ty in range(4):
        bias[:, ty] = np.tile(b1[32 * ty : 32 * ty + 32], 4)
        bias[:, 4 + ty] = np.tile(b_eff[32 * ty : 32 * ty + 32], 4)
    return dict(weff=weff, whh=whh, wih=wih, bias=bias,
                W_out=W_out, b_out=b_out)


def prep_state(arr, feat):
    """[B_LOC, feat] batch-major -> strip layout [128, CALL] bf16."""
    a = np.zeros((B_LOC, H), np.float32)
    a[:, :feat] = arr[:, :feat]
    a = a.reshape(GROUPS, 4, NB, H)           # [g, x, j, k]
    a = a.transpose(1, 3, 0, 2)               # [x, k, g, j]
    return np.ascontiguousarray(a.reshape(128, CALL).astype(bf16))


def build_nc():
    import concourse.bass as bass
    import concourse.bacc as bacc
    import concourse.mybir as mybir
    import concourse.tile as tile

    F32 = mybir.dt.float32
    BF16 = mybir.dt.bfloat16
    AF = mybir.ActivationFunctionType
    OP = mybir.AluOpType

    nc = bacc.Bacc("TRN2", target_bir_lowering=False, debug=False)
    hT_d = nc.declare_dram_parameter("hT", [128, CALL], BF16, isOutput=False)
    cT_d = nc.declare_dram_parameter("cT", [128, CALL], BF16, isOutput=False)
    xT_d = nc.declare_dram_parameter("xT", [128, CALL], BF16, isOutput=False)
    weff_d = nc.declare_dram_parameter("weff", [128, 512], BF16, isOutput=False)
    whh_d = nc.declare_dram_parameter("whh", [128, 512], BF16, isOutput=False)
    wih_d = nc.declare_dram_parameter("wih", [128, 512], BF16, isOutput=False)
    bias_d = nc.declare_dram_parameter("bias", [128, 8], F32, isOutput=False)
    hout_d = nc.declare_dram_parameter(
        "hout", [128, SEQ * BLOCKS * FD], BF16, isOutput=True)

    # wave order: f first (t1 needs it), then i, g (t2), o last
    WAVES = [(1, AF.Sigmoid), (0, AF.Sigmoid), (2, AF.Tanh), (3, AF.Sigmoid)]

    with tile.TileContext(nc) as tc:
        with (
            tc.tile_pool(name="const", bufs=1) as const,
            tc.tile_pool(name="hpool", bufs=2) as hpool,
            tc.tile_pool(name="gpsum", bufs=2, space=bass.MemorySpace.PSUM) as gpsum,
            tc.tile_pool(name="gate", bufs=3) as gate,
        ):
            weff_t = const.tile([128, 512], BF16)
            whh_t = const.tile([128, 512], BF16)
            wih_t = const.tile([128, 512], BF16)
            bias_t = const.tile([128, 8], F32)
            c_all = const.tile([128, CALL], BF16)
            # manually-recycled DVE scratch (poly temps + cell temps)
            sA = const.tile([128, FD2], BF16)
            sB = const.tile([128, FD2], BF16)
            sC = const.tile([128, FD2], BF16)
            sD = const.tile([128, FD2], BF16)
            sE = const.tile([128, FD2], BF16)
            nc.sync.dma_start(weff_t[:], weff_d[:])
            nc.sync.dma_start(whh_t[:], whh_d[:])
            nc.sync.dma_start(wih_t[:], wih_d[:])
            nc.sync.dma_start(bias_t[:], bias_d[:])

            # x0 rides the h ring: its slot is recycled for h(t=0) once
            # the step-0 matmuls have consumed it
            x0_t = hpool.tile([128, CALL], BF16, name="h")
            h_cur = hpool.tile([128, CALL], BF16, name="h")
            for hf in range(2):   # per-pair halves so step 0 starts sooner
                half = slice(hf * FD2, (hf + 1) * FD2)
                nc.sync.dma_start(c_all[:, half], cT_d[:, half])
                nc.sync.dma_start(x0_t[:, half], xT_d[:, half])
                nc.sync.dma_start(h_cur[:, half], hT_d[:, half])

            v = nc.vector

            def poly_tanh(dst, src, n):
                """dst = tanh(src[:, :n]) via clamped deg-7 odd poly."""
                xc, tt, z, u = sA, sB, sC, sD
                v.tensor_scalar(xc[:, :n], src, float(XCLAMP),
                                float(-XCLAMP), OP.min, OP.max)
                v.tensor_mul(tt[:, :n], xc[:, :n], xc[:, :n])
                v.tensor_scalar(z[:, :n], tt[:, :n], float(_C7),
                                float(GAMMA), OP.mult, OP.add)
                v.tensor_scalar_add(u[:, :n], tt[:, :n], float(ALPHA))
                v.tensor_mul(u[:, :n], u[:, :n], tt[:, :n])
                v.tensor_scalar_add(u[:, :n], u[:, :n], float(BETA))
                v.tensor_mul(tt[:, :n], u[:, :n], z[:, :n])
                v.tensor_scalar_add(tt[:, :n], tt[:, :n], float(DELTA))
                v.tensor_mul(dst, tt[:, :n], xc[:, :n])

            def finish_pair(t, pair, So, h_tile):
                """tanh(c'), h = sig(o)*tanh(c'), stream h out — for the
                pair whose cell update already ran (deferred one slot so
                the ACT tanh never head-of-line blocks fresh gate work)."""
                p0 = 2 * pair * FD
                cs = slice(p0, p0 + FD2)
                tc_t = sE
                if pair == 0:
                    poly_tanh(tc_t[:], c_all[:, cs], FD2)
                else:
                    nc.scalar.activation(tc_t[:], c_all[:, cs], AF.Tanh)
                v.tensor_mul(h_tile[:, cs], So[:], tc_t[:])
                nc.sync.dma_start(
                    hout_d[:, (t * BLOCKS + 2 * pair) * FD :
                           (t * BLOCKS + 2 * pair + 2) * FD],
                    h_tile[:, cs])

            pending = None
            for t in range(SEQ):
                h_next = hpool.tile([128, CALL], BF16, name="h")
                for pair in range(2):
                    p0 = 2 * pair * FD           # col offset of this pair
                    # gate tensors for this pair, [128, FD2] per type
                    S = {}
                    for ty, func in WAVES:
                        s_t = gate.tile([128, FD2], BF16, name=f"s{ty}")
                        ws = slice(128 * ty, 128 * ty + 128)
                        first_mm = True
                        for half, b in enumerate((2 * pair, 2 * pair + 1)):
                            P = gpsum.tile([128, FD], F32, name="P")
                            for g4 in range(4):
                                g = 4 * b + g4
                                gc = slice(NB * g, NB * (g + 1))
                                ps = P[:, NB * g4 : NB * (g4 + 1)]
                                if t == 0:
                                    nc.tensor.matmul(
                                        ps, whh_t[:, ws], h_cur[:, gc],
                                        start=True, stop=False)
                                    nc.tensor.matmul(
                                        ps, wih_t[:, ws], x0_t[:, gc],
                                        start=False, stop=True)
                                else:
                                    # the 8 matmuls of one (type, pair)
                                    # group share the stationary — load
                                    # the PE array once
                                    mm = nc.tensor.matmul(
                                        ps, weff_t[:, ws], h_cur[:, gc],
                                        start=True, stop=True)
                                    if not first_mm:
                                        mm.ldweights = False
                                    first_mm = False
                            bcol = ty if t == 0 else 4 + ty
                            nc.scalar.activation(
                                s_t[:, half * FD : (half + 1) * FD], P[:],
                                func, bias=bias_t[:, bcol : bcol + 1])
                        S[ty] = s_t

                    # finish the PREVIOUS pair (its tanh/h/dma) now that
                    # this pair's gate ACTs are queued ahead of the tanh
                    if pending is not None:
                        finish_pair(*pending)
                        pending = None

                    # cell update at FD2=4096 (scratch sA/sB recycled)
                    cs = slice(p0, p0 + FD2)
                    t1, t2 = sA, sB
                    v.tensor_mul(t1[:], S[1][:], c_all[:, cs])
                    v.tensor_mul(t2[:], S[0][:], S[2][:])
                    v.tensor_add(c_all[:, cs], t1[:], t2[:])

                    pending = (t, pair, S[3], h_next)
                # pair0 h-cols finish during this step's pair1 slot; pair1
                # h-cols finish during next step's pair0 slot — both before
                # the matmuls that read them
                h_cur = h_next
            finish_pair(*pending)
    nc.compile()
    return nc


_NC_CACHE = {}


def _get_nc(key="v3"):
    if key not in _NC_CACHE:
        _NC_CACHE[key] = build_nc()
    return _NC_CACHE[key]


def make_in_maps(inputs):
    first_input = _f32(inputs["first_input"])
    h0 = _f32(inputs["h0"])
    c0 = _f32(inputs["c0"])
    w = prep_weights(
        inputs["W_ih"], inputs["W_hh"], inputs["b_ih"], inputs["b_hh"],
        inputs["W_out"], inputs["b_out"],
    )
    shared = dict(weff=w["weff"], whh=w["whh"], wih=w["wih"], bias=w["bias"])
    in_maps = []
    for ci in range(NCORES):
        rows = slice(ci * B_LOC, (ci + 1) * B_LOC)
        in_maps.append(dict(
            shared,
            hT=prep_state(h0[rows], H),
            cT=prep_state(c0[rows], H),
            xT=prep_state(first_input[rows], PD),
        ))
    return in_maps, w


def postprocess(results, w):
    """Per-core hout [128, SEQ*BLOCKS*FD] bf16 -> full [BATCH, SEQ, PD] f32."""
    W_outT = w["W_out"].T.astype(np.float32)       # [H, PD]
    b_out = w["b_out"].astype(np.float32)
    outs = []
    for ci in range(NCORES):
        a = np.asarray(results[ci]["hout"])
        # [128, SEQ*BLOCKS*FD] -> [x, k, t, b, g4, j]
        a = a.reshape(4, 32, SEQ, BLOCKS, 4, NB)
        # -> [b, g4, x, j, t, k]
        a = np.ascontiguousarray(a.transpose(3, 4, 0, 5, 2, 1))
        h = a.reshape(B_LOC * SEQ, H).astype(np.float32)
        x = h @ W_outT + b_out
        outs.append(x.reshape(B_LOC, SEQ, PD))
    return np.concatenate(outs, axis=0)


def kernel(**inputs) -> np.ndarray:
    from concourse.bass_utils import run_bass_kernel_spmd

    in_maps, w = make_in_maps(inputs)
    nc = _get_nc()
    res = run_bass_kernel_spmd(nc, in_maps, core_ids=list(range(NCORES)))
    return postprocess(res.results, w)


if __name__ == "__main__":
    nc = build_nc()
    n = sum(len(b.instructions) for b in nc.m.functions[0].blocks)
    print("built; instructions:", n)
